# revision 3
# baseline (speedup 1.0000x reference)
"""TRN2 Bass kernel v2 for nn_GraphVectorEncoder (3x TransformerConv + mean pool).

Design (per core, nodes partitioned by contiguous dst ranges, degree-sorted
into 128-row tiles with K edge slots):
  node phase l: hT = transpose(slab tile); [q|k|v|s|c] = hT @ Wbig + brow;
                kv16 tile -> kv_bounce DRAM.
  chunked AllGather kv_bounce -> kv_full [NPAD, 256] bf16 (k~ and v~ tables).
  edge phase l: one indirect DMA gathers [k~|v~] rows for all K slots;
                w = k~ (*) q~ (DVE 2x bf16); alpha = binary-tree reduce;
                alpha' = alpha + c + mask; pexp = Exp(alpha'-amax) w/ fused
                denominator (Act); pw = broadcast(pexp) (Act); M = v~ (*) pw
                (DVE 2x); agg = sum_j M_j via identity-stationary PSUM matmuls
                (PE); out_h = agg_h * rec_h + skip (DVE); relu -> slab (Act).
  Mean pool: one-hot matmul accumulation (PE), host combines cores.
"""

import numpy as np
import ml_dtypes

N, E, G = 50000, 800000, 64
D = 128
ROW = 2 * D
NC = 8
NLOC = N // NC             # 6250
TILES = (NLOC + 127) // 128  # 49
PADLOC = TILES * 128       # 6272
NPAD = NC * PADLOC         # 50176
LAYER_HEADS = [2, 2, 1]
LAYER_HD = [64, 64, 128]
NCHUNK = 1
BOUNDS = [0, TILES]

_CACHE = {}


def _build(meta):
    import concourse.bass as bass
    import concourse.mybir as mybir
    import concourse.tile as tile
    from concourse import bacc
    from concourse.masks import make_identity

    Kt = meta["Kt"]
    SLOTS = int(Kt.sum())
    off = np.concatenate([[0], np.cumsum(Kt)]).astype(int)
    KMAX = int(Kt.max())

    nc = bacc.Bacc("TRN2", target_bir_lowering=False, debug=False, num_devices=NC)
    f32 = mybir.dt.float32
    bf16 = mybir.dt.bfloat16
    i32 = mybir.dt.int32

    xin = nc.dram_tensor("xin", [128, TILES * 128], bf16, kind="ExternalInput")
    kvin = nc.dram_tensor("kvin", [NPAD, ROW], bf16, kind="ExternalInput")
    gidx = nc.dram_tensor("gidx", [128, SLOTS], i32, kind="ExternalInput")
    mbig = nc.dram_tensor("mbig", [128, SLOTS], bf16, kind="ExternalInput")
    oneh = nc.dram_tensor("oneh", [128, TILES * G], f32, kind="ExternalInput")
    Fs = [4 * D + h for h in LAYER_HEADS]
    wbs, wbds, brs = [], [], []
    for li in range(3):
        wbs.append(nc.dram_tensor(f"wb{li}", [128, Fs[li]], bf16, kind="ExternalInput"))
        wbds.append(nc.dram_tensor(f"wbd{li}", [128, Fs[li]], bf16, kind="ExternalInput"))
        brs.append(nc.dram_tensor(f"br{li}", [128, Fs[li]], bf16, kind="ExternalInput"))
    pool_out = nc.dram_tensor("pool_out", [G, D], f32, kind="ExternalOutput")

    kv_bounce = nc.dram_tensor("kv_bounce", [PADLOC, ROW], bf16)
    kv_fulls = [kvin,
                nc.dram_tensor("kv_full1", [NPAD, ROW], bf16),
                nc.dram_tensor("kv_full2", [NPAD, ROW], bf16)]

    bounds = BOUNDS

    with tile.TileContext(nc) as tc:
        with (
            tc.tile_pool(name="const", bufs=1) as cp,
            tc.tile_pool(name="nps", bufs=2, space="PSUM") as npp,
            tc.tile_pool(name="npsq", bufs=1, space="PSUM") as npq,
            tc.tile_pool(name="gw", bufs=3) as gp,
            tc.tile_pool(name="wp", bufs=2) as wp,
            tc.tile_pool(name="mp", bufs=2) as mp,
            tc.tile_pool(name="tp", bufs=1) as tp,
            tc.tile_pool(name="rp", bufs=3) as rp,
            tc.tile_pool(name="ps2", bufs=2, space="PSUM") as p2p,
            tc.tile_pool(name="pool_ps", bufs=1, space="PSUM") as ppool,
        ):
            identb = cp.tile([128, 128], bf16)
            make_identity(nc, identb[:])
            identf = cp.tile([128, 128], f32)
            make_identity(nc, identf[:])
            gidx_sb = cp.tile([128, SLOTS], i32)
            nc.sync.dma_start(gidx_sb[:], gidx[:])
            mbig_sb = cp.tile([128, SLOTS], bf16)
            nc.sync.dma_start(mbig_sb[:], mbig[:])
            oneh_sb = cp.tile([128, TILES * G], f32)
            nc.sync.dma_start(oneh_sb[:], oneh[:])
            wb_sb, wbd_sb, br_sb = [], [], []
            for li in range(3):
                w = cp.tile([128, Fs[li]], bf16, name=f"wb_sb{li}")
                nc.sync.dma_start(w[:], wbs[li][:])
                wb_sb.append(w)
                wd = cp.tile([128, Fs[li]], bf16, name=f"wbd_sb{li}")
                nc.sync.dma_start(wd[:], wbds[li][:])
                wbd_sb.append(wd)
                b = cp.tile([128, Fs[li]], bf16, name=f"br_sb{li}")
                nc.sync.dma_start(b[:], brs[li][:])
                br_sb.append(b)

            # slabs hold hT: column t*128+i is node i of tile t, partition=d
            slabA = cp.tile([128, TILES * 128], bf16)
            nc.sync.dma_start(slabA[:], xin[:])
            slabB = cp.tile([128, TILES * 128], bf16)
            Q16 = cp.tile([128, TILES * D], bf16)
            S16 = cp.tile([128, TILES * D], bf16)
            C32 = cp.tile([128, TILES * 2], f32)

            pool_ps = ppool.tile([G, D], f32, space="PSUM")

            def node_tile(li, t, slab_in):
                F = Fs[li]
                H = LAYER_HEADS[li]
                qk_ps = npq.tile([128, 4 * D], f32, space="PSUM", name="qk_ps")
                lhs = slab_in[:, t * 128:(t + 1) * 128]
                nc.tensor.matmul(qk_ps[:], lhsT=lhs,
                                 rhs=wb_sb[li][:, :4 * D],
                                 start=True, stop=False)
                nc.tensor.matmul(qk_ps[:], lhsT=lhs,
                                 rhs=wbd_sb[li][:, :4 * D],
                                 start=False, stop=True)
                c_ps = npp.tile([128, 2], f32, space="PSUM", name="c_ps")
                nc.tensor.matmul(c_ps[:, :H], lhsT=lhs,
                                 rhs=wb_sb[li][:, 4 * D:4 * D + H],
                                 start=True, stop=False)
                nc.tensor.matmul(c_ps[:, :H], lhsT=lhs,
                                 rhs=wbd_sb[li][:, 4 * D:4 * D + H],
                                 start=False, stop=True)
                # bias adds straight into their destination buffers
                nc.vector.scalar_tensor_tensor(
                    out=Q16[:, t * D:(t + 1) * D], in0=qk_ps[:, :D],
                    scalar=1.0, in1=br_sb[li][:, :D],
                    op0=mybir.AluOpType.mult, op1=mybir.AluOpType.add)
                nc.vector.scalar_tensor_tensor(
                    out=S16[:, t * D:(t + 1) * D], in0=qk_ps[:, 3 * D:4 * D],
                    scalar=1.0, in1=br_sb[li][:, 3 * D:4 * D],
                    op0=mybir.AluOpType.mult, op1=mybir.AluOpType.add)
                nc.vector.scalar_tensor_tensor(
                    out=C32[:, t * 2:t * 2 + H], in0=c_ps[:, :H],
                    scalar=1.0, in1=br_sb[li][:, 4 * D:4 * D + H],
                    op0=mybir.AluOpType.mult, op1=mybir.AluOpType.add)
                if li > 0:
                    kv16 = wp.tile([128, 2 * D], bf16, name="kv16")
                    nc.vector.scalar_tensor_tensor(
                        out=kv16[:], in0=qk_ps[:, D:3 * D],
                        scalar=1.0, in1=br_sb[li][:, D:3 * D],
                        op0=mybir.AluOpType.mult, op1=mybir.AluOpType.add)
                    nc.gpsimd.dma_start(
                        kv_bounce[t * 128:(t + 1) * 128, :],
                        kv16[:])

            def edge_A(li, t):
                """gather, alpha, softmax, pexp broadcast. Returns ctx."""
                H = LAYER_HEADS[li]
                hd = LAYER_HD[li]
                K = int(Kt[t])
                o0 = int(off[t])
                if K == 0:
                    return None
                Gt = gp.tile([128, KMAX * ROW], bf16, name="Gt")
                nc.gpsimd.indirect_dma_start(
                    out=Gt[:, :K * ROW], out_offset=None,
                    in_=kv_fulls[li][:],
                    in_offset=bass.IndirectOffsetOnAxis(
                        ap=gidx_sb[:, o0:o0 + K], axis=0))
                # w = ktil (*) qtil  (both heads at once)
                w = wp.tile([128, KMAX * D], bf16, name="w")
                in0 = bass.AP(Gt.tensor, Gt[:].offset,
                              [Gt[:].ap[0], [ROW, K], [1, D]])
                in1 = bass.AP(Q16.tensor, Q16[:].offset + t * D,
                              [Q16[:].ap[0], [0, K], [1, D]])
                nc.vector.tensor_tensor(
                    out=bass.AP(w.tensor, w[:].offset,
                                [w[:].ap[0], [D, K], [1, D]]),
                    in0=in0, in1=in1, op=mybir.AluOpType.mult)
                # tree reduce over hd within each head -> alpha [p, K, H]
                src, src_hd = w, hd
                n = hd // 2
                while n >= 1:
                    dt_ = bf16 if n > 4 else f32
                    dstt = tp.tile([128, KMAX * H * max(n, 1)], dt_,
                                   name=f"tree{n}")
                    i0 = bass.AP(src.tensor, src[:].offset,
                                 [src[:].ap[0], [H * src_hd, K],
                                  [src_hd, H], [1, n]])
                    i1 = bass.AP(src.tensor, src[:].offset + n,
                                 [src[:].ap[0], [H * src_hd, K],
                                  [src_hd, H], [1, n]])
                    nc.vector.tensor_tensor(
                        out=bass.AP(dstt.tensor, dstt[:].offset,
                                    [dstt[:].ap[0], [H * n, K],
                                     [n, H], [1, n]]),
                        in0=i0, in1=i1, op=mybir.AluOpType.add)
                    src, src_hd = dstt, n
                    n //= 2
                # alpha' = (tree + c_h) + mask ; amax ; pexp ; den ; rec
                alp = wp.tile([128, H * KMAX], f32, name="alp")
                nmax = wp.tile([128, 2], f32, name="nmax")
                pexp = wp.tile([128, H * KMAX], bf16, name="pexp")
                den = wp.tile([128, 2], f32, name="den")
                rec = rp.tile([128, 2], f32, name="rec")
                for h in range(H):
                    nc.vector.scalar_tensor_tensor(
                        out=alp[:, h * K:(h + 1) * K],
                        in0=bass.AP(src.tensor, src[:].offset + h,
                                    [src[:].ap[0], [H, K]]),
                        scalar=C32[:, t * 2 + h:t * 2 + h + 1],
                        in1=mbig_sb[:, o0:o0 + K],
                        op0=mybir.AluOpType.add, op1=mybir.AluOpType.add)
                    nc.vector.tensor_reduce(
                        out=nmax[:, h:h + 1], in_=alp[:, h * K:(h + 1) * K],
                        axis=mybir.AxisListType.X, op=mybir.AluOpType.max)
                    nc.vector.tensor_scalar(
                        out=alp[:, h * K:(h + 1) * K],
                        in0=alp[:, h * K:(h + 1) * K],
                        scalar1=nmax[:, h:h + 1], scalar2=None,
                        op0=mybir.AluOpType.subtract)
                    nc.scalar.activation(
                        pexp[:, h * K:(h + 1) * K],
                        alp[:, h * K:(h + 1) * K],
                        mybir.ActivationFunctionType.Exp)
                    nc.vector.tensor_reduce(
                        out=den[:, h:h + 1], in_=pexp[:, h * K:(h + 1) * K],
                        axis=mybir.AxisListType.X, op=mybir.AluOpType.add)
                nc.vector.tensor_scalar(out=den[:, :H], in0=den[:, :H],
                                        scalar1=1e-16, scalar2=None,
                                        op0=mybir.AluOpType.add)
                nc.vector.reciprocal(out=rec[:, :H], in_=den[:, :H])
                # pw = broadcast(pexp) on Act
                pw = mp.tile([128, KMAX * D], bf16, name="pw")
                nc.scalar.activation(
                    bass.AP(pw.tensor, pw[:].offset,
                            [pw[:].ap[0], [D, K], [hd, H], [1, hd]]),
                    bass.AP(pexp.tensor, pexp[:].offset,
                            [pexp[:].ap[0], [1, K], [K, H], [0, hd]]),
                    mybir.ActivationFunctionType.Copy)
                return (Gt, pw, rec)

            def edge_M(li, t, ctx):
                """M = vtil (*) pw — separate stage so Gt frees early."""
                if ctx is None:
                    return None
                K = int(Kt[t])
                Gt, pw, rec = ctx
                M = mp.tile([128, KMAX * D], bf16, name="M")
                nc.vector.tensor_tensor(
                    out=bass.AP(M.tensor, M[:].offset,
                                [M[:].ap[0], [D, K], [1, D]]),
                    in0=bass.AP(Gt.tensor, Gt[:].offset + D,
                                [Gt[:].ap[0], [ROW, K], [1, D]]),
                    in1=bass.AP(pw.tensor, pw[:].offset,
                                [pw[:].ap[0], [D, K], [1, D]]),
                    op=mybir.AluOpType.mult)
                return (M, rec)

            def edge_B(li, t, ctx, slab_out):
                H = LAYER_HEADS[li]
                hd = LAYER_HD[li]
                K = int(Kt[t])
                if ctx is None:
                    ot = wp.tile([128, D], f32, name="ot")
                    nc.vector.tensor_copy(out=ot[:],
                                          in_=S16[:, t * D:(t + 1) * D])
                else:
                    M, rec = ctx
                    # agg = sum_j M_j (identity-stationary PSUM accumulation)
                    ps2 = p2p.tile([128, D], f32, space="PSUM", name="ps2")
                    for j in range(K):
                        nc.tensor.matmul(ps2[:], lhsT=identb[:],
                                         rhs=M[:, j * D:(j + 1) * D],
                                         start=(j == 0), stop=(j == K - 1))
                    # out = agg_h * rec_h + skip
                    ot = wp.tile([128, D], f32, name="ot")
                    for h in range(H):
                        nc.vector.scalar_tensor_tensor(
                            out=ot[:, h * hd:(h + 1) * hd],
                            in0=ps2[:, h * hd:(h + 1) * hd],
                            scalar=rec[:, h:h + 1],
                            in1=S16[:, t * D + h * hd:t * D + (h + 1) * hd],
                            op0=mybir.AluOpType.mult, op1=mybir.AluOpType.add)
                if li < 2:
                    otT = npp.tile([128, D], f32, space="PSUM", name="otT")
                    nc.tensor.transpose(out=otT[:], in_=ot[:],
                                        identity=identf[:])
                    nc.scalar.activation(slab_out[:, t * 128:(t + 1) * 128],
                                         otT[:],
                                         mybir.ActivationFunctionType.Relu)
                else:
                    hf = wp.tile([128, D], f32, name="hf")
                    nc.scalar.activation(hf[:], ot[:],
                                         mybir.ActivationFunctionType.Relu)
                    nc.tensor.matmul(
                        pool_ps[:], lhsT=oneh_sb[:, t * G:(t + 1) * G],
                        rhs=hf[:], start=(t == 0), stop=(t == TILES - 1))

            def allgather_chunk(li, ci):
                t0, t1 = bounds[ci], bounds[ci + 1]
                r0, r1 = t0 * 128, t1 * 128
                ins = bass.AP(kv_bounce, r0 * ROW,
                              [[1, (r1 - r0) * ROW]])
                # chunk-major table layout: replicas of chunk ci are
                # contiguous at rows [NC*r0, NC*r1); 2D AP (replica dim
                # first) is still byte-contiguous
                outs = bass.AP(kv_fulls[li], NC * r0 * ROW,
                               [[(r1 - r0) * ROW, NC], [1, (r1 - r0) * ROW]])
                nc.gpsimd.collective_compute(
                    "AllGather", mybir.AluOpType.bypass,
                    replica_groups=[list(range(NC))],
                    ins=[ins], outs=[outs])

            slabs = [(slabA, slabB), (slabB, slabA), (slabA, slabB)]
            # layer-1 node phase (kv table for layer 1 comes from the host)
            for t in range(TILES):
                node_tile(0, t, slabs[0][0])
            # edge(li) software-pipelined 3 deep: A(t+2) | M(t+1) | B(t),
            # with node(li+1) + chunked allgather(li+1) folded into B
            chunk_end = {bounds[ci + 1] - 1: ci for ci in range(NCHUNK)}
            for li in range(3):
                slab_in, slab_out = slabs[li]

                def do_B(t):
                    edge_B(li, t, mctxs.pop(t), slab_out)
                    if li < 2:
                        node_tile(li + 1, t, slab_out)
                        if t in chunk_end:
                            allgather_chunk(li + 1, chunk_end[t])

                ctxs, mctxs = {}, {}
                for t in range(TILES):
                    ctxs[t] = edge_A(li, t)
                    if t >= 1:
                        mctxs[t - 1] = edge_M(li, t - 1, ctxs.pop(t - 1))
                    if t >= 2:
                        do_B(t - 2)
                mctxs[TILES - 1] = edge_M(li, TILES - 1, ctxs.pop(TILES - 1))
                do_B(TILES - 2)
                do_B(TILES - 1)

            pout_sb = cp.tile([G, D], f32)
            nc.vector.tensor_copy(out=pout_sb[:], in_=pool_ps[:])
            nc.sync.dma_start(pool_out[:], pout_sb[:])
    nc.compile()
    return nc


def _prep(x, edge_index, batch, weights):
    src = np.asarray(edge_index[0], dtype=np.int64)
    dst = np.asarray(edge_index[1], dtype=np.int64)
    batch = np.asarray(batch, dtype=np.int64)
    deg = np.bincount(dst, minlength=N)

    order = np.argsort(dst, kind="stable")
    src_sorted = src[order]
    starts = np.concatenate([[0], np.cumsum(deg)]).astype(np.int64)
    # rank of each sorted edge within its dst segment
    rank = np.arange(E, dtype=np.int64) - starts[dst[order]]

    perm = np.zeros((NC, PADLOC), dtype=np.int64)
    degs = np.zeros((NC, PADLOC), dtype=np.int64)
    valid = np.zeros((NC, PADLOC), dtype=bool)
    ipos_global = np.zeros(N, dtype=np.int64)  # row of node in padded table
    for c in range(NC):
        ids = np.arange(c * NLOC, (c + 1) * NLOC)
        o = np.argsort(-deg[ids], kind="stable")
        perm[c, :NLOC] = ids[o]
        degs[c, :NLOC] = deg[ids][o]
        valid[c, :NLOC] = True
        ipos_global[ids[o]] = c * PADLOC + np.arange(NLOC)

    Kt = degs.reshape(NC, TILES, 128).max(axis=(0, 2)).astype(np.int64)
    SLOTS = int(Kt.sum())
    off = np.concatenate([[0], np.cumsum(Kt)]).astype(np.int64)

    # chunk-major table row for each node (matches allgather output layout)
    rowb = np.array([b * 128 for b in BOUNDS], dtype=np.int64)
    iloc = ipos_global % PADLOC
    icore = ipos_global // PADLOC
    ci = np.searchsorted(rowb, iloc, side="right") - 1
    r0 = rowb[ci]
    r1 = rowb[ci + 1]
    trow = NC * r0 + icore * (r1 - r0) + (iloc - r0)

    ins = []
    for c in range(NC):
        # edges with dst in this core, in sorted order
        emask = (dst[order] >= c * NLOC) & (dst[order] < (c + 1) * NLOC)
        ed = dst[order][emask]
        es = src_sorted[emask]
        er = rank[emask]
        i = ipos_global[ed] - c * PADLOC       # position in core slab
        tt = i // 128
        pp_ = i % 128
        flat = pp_ * SLOTS + off[tt] + er
        gi = np.zeros(128 * SLOTS, dtype=np.int32)
        mb = np.full(128 * SLOTS, -30.0, dtype=np.float32)
        gi[flat] = trow[es].astype(np.int32)
        mb[flat] = 0.0
        gi = gi.reshape(128, SLOTS)
        mb = mb.reshape(128, SLOTS)

        ohb = np.zeros((128, TILES * G), dtype=np.float32)
        pv = perm[c].reshape(TILES, 128)
        vv = valid[c].reshape(TILES, 128)
        tgrid, pgrid = np.nonzero(vv)
        ohb[pgrid, tgrid * G + batch[pv[tgrid, pgrid]]] = 1.0

        xp = np.zeros((PADLOC, D), dtype=np.float32)
        xp[:NLOC] = np.asarray(x, dtype=np.float32)[perm[c, :NLOC]]
        m = dict(xin=np.ascontiguousarray(xp.T).astype(ml_dtypes.bfloat16),
                 gidx=gi,
                 mbig=mb.astype(ml_dtypes.bfloat16),
                 oneh=ohb)
        ins.append(m)

    # host-computed layer-1 [ktil|vtil] table in padded-permuted layout
    Wq1, bq1, Wk1, bk1, Wv1, bv1, Ws1, bs1 = weights[0]
    s1 = 1.0 / np.sqrt(LAYER_HD[0])
    xf = np.asarray(x, dtype=np.float32)
    kv1 = np.zeros((NPAD, ROW), dtype=np.float32)
    kv1[trow, :D] = xf @ (Wk1 * s1)
    kv1[trow, D:] = xf @ Wv1 + bv1[None, :]
    kv1 = kv1.astype(ml_dtypes.bfloat16)
    for m in ins:
        m["kvin"] = kv1

    for li in range(3):
        H = LAYER_HEADS[li]
        hd = LAYER_HD[li]
        Wq, bq, Wk, bk, Wv, bv, Ws, bs = weights[li]
        din = Wq.shape[0]
        s = 1.0 / np.sqrt(hd)
        F = 4 * D + H
        wb = np.zeros((128, F), dtype=np.float32)
        br = np.zeros((128, F), dtype=np.float32)
        wb[:, :D] = Wq
        br[:, :D] = bq[None, :]
        wb[:, D:2 * D] = Wk * s
        wb[:, 2 * D:3 * D] = Wv
        br[:, 2 * D:3 * D] = bv[None, :]
        wb[:, 3 * D:4 * D] = Ws
        br[:, 3 * D:4 * D] = bs[None, :]
        Wq_h = Wq.reshape(din, H, hd)
        bq_h = bq.reshape(H, hd)
        bk_h = bk.reshape(H, hd)
        for h in range(H):
            wb[:, 4 * D + h] = (Wq_h[:, h] @ bk_h[h]) * s
            br[:, 4 * D + h] = float(bq_h[h] @ bk_h[h]) * s
        wb16 = wb.astype(ml_dtypes.bfloat16)
        wbd16 = (wb - wb16.astype(np.float32)).astype(ml_dtypes.bfloat16)
        for m in ins:
            m[f"wb{li}"] = wb16
            m[f"wbd{li}"] = wbd16
            m[f"br{li}"] = br.astype(ml_dtypes.bfloat16)
    return ins, Kt, batch


def kernel(**inputs):
    x = np.asarray(inputs["x"], dtype=np.float32)
    weights = []
    for li in range(1, 4):
        weights.append(tuple(np.asarray(inputs[f"{nm}{li}"], dtype=np.float32)
                             for nm in ("Wq", "bq", "Wk", "bk", "Wv", "bv", "Ws", "bs")))
    ins, Kt, batch = _prep(x, inputs["edge_index"], inputs["batch"], weights)

    key = tuple(Kt.tolist())
    if key not in _CACHE:
        _CACHE[key] = _build({"Kt": Kt})
    nc = _CACHE[key]

    from concourse.bass_utils import run_bass_kernel_spmd
    r = run_bass_kernel_spmd(nc, ins, core_ids=list(range(NC)))
    parts = np.stack([r.results[c]["pool_out"] for c in range(NC)])
    sums = parts.sum(axis=0)
    cnts = np.bincount(np.asarray(batch, dtype=np.int64), minlength=G).astype(np.float32)
    return (sums / np.maximum(cnts, 1.0)[:, None]).astype(np.float32)
